# revision 16
# baseline (speedup 1.0000x reference)
"""Trainium2 Bass kernel for nn_DiseaseClassifier (segment_reduce).

reference semantics:
    m = mask.astype(f32); counts = m.sum(0)
    pooled = einsum('brh,rd->bdh', x, m) / max(counts,1)
    h = einsum('bdh,dhk->bdk', pooled, W1) + b1
    hn = LN(h) * gamma + beta ; g = gelu_exact(hn)
    preds = einsum('bdk,dk->bd', g, W2) + b2 ; preds[counts==0] = 0

Key algebraic facts used:
  * LayerNorm is scale-invariant, so the 1/count pooling divisor cancels
    (when b1 != 0 we add counts*b1 to the un-normalized pool-matmul output,
    which keeps the invariance exact).
  * b2 rides on the host; the counts==0 zeroing folds into W2.

Precision plan (correctness gate is rel err < 2e-2; measured 2.9e-3 in
host simulation): x, W1, pooled, gelu output and W2 all in bf16; all
matmul accumulation in f32 PSUM.  This halves the dominant HBM traffic
(x: 24.6 MB -> one bf16 plane) vs the fp32-accurate hi/lo bf16 split,
halves the phase-A stationary loads, and makes every matmul FWL-eligible
(128-col bf16 stationary => 2x weight-load bandwidth, load overlapped
with the previous matmul's moving phase; fp32/fp32r stationary loads
serialize and get no FWL).

Distribution: batch dim sharded over 8 NeuronCores (512 rows each); all
parameters replicated.  Inside each core:
  phase A: pool-matmul.  Stationary = x tiles [(4b,29r)=116, 128h] bf16,
           moving = 0/1 block-diag mask [116, 56=(14d,4j)] -> PSUM
           [128h, (2 groups)x(6hc,14d,4j)], evacuated by ScalarE/VectorE
           (alternating, 2 groups per instruction) into bf16
           pooledT [h, hc, d, b] layout.
  phase B: per-disease bf16 matmul pooledT[128h,128b] x W1[d][128h,384k]
           (6 h-chunks accumulated in PSUM), then bn_stats/bn_aggr ->
           Rsqrt -> single ScalarE activation gelu(scale*h+bias) with
           per-partition scale/bias doing the whole LayerNorm, then the
           g.W2 dot product on GpSimd (scalar_tensor_tensor with
           accum_out), keeping DVE free for stats + evac.

Engine budget per core-loop (modeled): PE ~100us (phase A 768 ldweights-
bound MMs + phase B 336 N=384 MMs), DMA ~69us (24.6MB bf16 x), ACT/DVE
~70us each (evac split + gelu / bn stats), GpSimd ~27us (dots).
"""

import os
import sys
import functools

for _p in ("/opt/trn_rl_repo", "/opt/pypackages"):
    if os.path.isdir(_p) and _p not in sys.path:
        sys.path.insert(0, _p)

import numpy as np

B, R, H, D = 4096, 29, 768, 14
K = H // 2            # 384
LN_EPS = 1e-5
NCORES = 8
BC = B // NCORES      # 512 batch rows per core
NCHUNK = BC // 128    # 4 chunks of 128 rows
NG = 32               # (4b,29r) groups per chunk
GB = 4                # groups per x-DMA batch
HC = H // 128         # 6 contraction chunks
JR = 4 * R            # 116 partitions for the pool matmul
DJ = D * 4            # 56 moving columns of the pool matmul


def _install_walrus_patches():
    """This walrus build supports only ONE sem wait per instruction
    ("Too many sync wait commands").  Split Tile-assigned multi-waits onto
    same-engine NoOps placed right before the instruction, and do the same
    for the TileContext tail drain."""
    from concourse import tile as _tile
    from concourse import mybir
    from concourse.vector_clock import ScopedClock

    if getattr(_tile.TileContext, "_ant_wait_split_patch", False):
        return
    _orig_commit = _tile.TileContext._commit_instruction

    def _patched_commit(self, inst, lazy_reg_writes=True):
        si = getattr(inst, "sync_info", None)
        if si is not None and si.on_wait and len(si.on_wait) > 1:
            waits = list(si.on_wait)
            inst.sync_info = mybir.SyncInfo(
                on_wait=[waits[-1]], on_update=list(si.on_update or [])
            )
            for w in waits[:-1]:
                nop = mybir.InstNoOp(
                    name=self.nc.get_next_instruction_name(), ins=[], outs=[]
                )
                nop.engine = inst.engine
                nop.sync_info = mybir.SyncInfo(on_wait=[w], on_update=[])
                self._add_instruction(nop)
        return _orig_commit(self, inst, lazy_reg_writes)

    def _patched_drain_and_barrier(self, tick_clock, wait_clock):
        drain_inst = self.nc.sync.drain()
        wait_clock.add_sem_waits(
            drain_inst.ins, ScopedClock({None: tick_clock.global_clock})
        )
        si = drain_inst.ins.sync_info
        if si is not None and si.on_wait and len(si.on_wait) > 1:
            waits = list(si.on_wait)
            drain_inst.ins.sync_info = mybir.SyncInfo(
                on_wait=[waits[0]], on_update=list(si.on_update or [])
            )
            for w in waits[1:]:
                d2 = self.nc.sync.drain()
                d2.ins.sync_info = mybir.SyncInfo(on_wait=[w], on_update=[])
        self.nc.all_engine_barrier()
        assert self.sems is not None
        popped = self.nc._tile_sem_poison_stack.pop()
        assert popped is self._sem_poison
        self.nc.clear_and_free_semaphores(list(self.sems.allocated().values()))
        self.nc.all_engine_barrier()

    _tile.TileContext._commit_instruction = _patched_commit
    _tile.TileContext._drain_and_barrier = _patched_drain_and_barrier
    _tile.TileContext._ant_wait_split_patch = True


@functools.lru_cache(maxsize=16)
def build_nc(with_b1: bool = False, with_affine: bool = False, repeat: int = 1,
             variant: str = "full", SG: int = 2, ev_act: int = 4,
             dot_gp: bool = True, cp_act: int = 0, ilv: bool = True):
    """Build the Bass program (identical on all 8 cores).

    ev_act: of every 8 two-group evacuations, this many go to ScalarE
    (the rest to VectorE).  cp_act: of every 8 hps->SBUF copies, this many
    go to ScalarE.  dot_gp: dot product = GpSimd mult + DVE reduce (else
    one DVE tensor_tensor_reduce)."""
    import concourse.bass as bass
    import concourse.mybir as mybir
    from concourse.tile import TileContext

    _install_walrus_patches()

    F32 = mybir.dt.float32
    F32R = mybir.dt.float32r
    BF16 = mybir.dt.bfloat16
    AF = mybir.ActivationFunctionType
    ALU = mybir.AluOpType

    nc = bass.Bass("TRN2", target_bir_lowering=False, debug=False,
                   num_devices=NCORES)

    x = nc.declare_dram_parameter("x", [NCHUNK, NG // GB, 128, GB * H],
                                  BF16, isOutput=False)
    # mask padded to 128 contraction rows (rows 116..127 zero): keeps the
    # stationary x tiles at the full 128 partitions so FWL triggers.
    mblk = nc.declare_dram_parameter("mblk", [128, DJ], BF16, isOutput=False)
    w1t = nc.declare_dram_parameter("w1t", [128, D * HC * K], BF16,
                                    isOutput=False)
    w2rep = nc.declare_dram_parameter("w2rep", [128, D * K], BF16,
                                      isOutput=False)
    if with_b1:
        b1x = nc.declare_dram_parameter("b1x", [1, D * K], F32R, isOutput=False)
    if with_affine:
        garep = nc.declare_dram_parameter("garep", [128, D, K], F32, isOutput=False)
        berep = nc.declare_dram_parameter("berep", [128, D, K], F32, isOutput=False)
    out = nc.declare_dram_parameter("out", [128, NCHUNK * D], F32, isOutput=True)

    with TileContext(nc) as tc:
        with (
            tc.tile_pool(name="const", bufs=1) as constp,
            tc.tile_pool(name="xin", bufs=6) as xp,
            tc.tile_pool(name="gly", bufs=3) as gp,
            tc.tile_pool(name="hb", bufs=32) as hbp,
            tc.tile_pool(name="st", bufs=4) as stp,
            tc.tile_pool(name="pg", bufs=2, space="PSUM") as pgp,
            tc.tile_pool(name="hp", bufs=4, space="PSUM") as hpp,
        ):
            mb = constp.tile([128, DJ], BF16, tag="mblk")
            nc.sync.dma_start(out=mb[:], in_=mblk[:])
            w1sb = constp.tile([128, D, HC, K], BF16, tag="w1sb")
            nc.sync.dma_start(
                out=w1sb.rearrange("p d hc k -> p (d hc k)"), in_=w1t[:])
            w2sb = constp.tile([128, D, K], BF16, tag="w2sb")
            nc.sync.dma_start(
                out=w2sb.rearrange("p d k -> p (d k)"), in_=w2rep[:])
            # double-buffered per chunk: breaks the WAR serialization
            # (phase-B reads of chunk c vs phase-A evac writes of chunk c+1)
            pts = [constp.tile([128, HC, D, 128], BF16, tag=f"pt{i}",
                               name=f"pt{i}") for i in range(2)]

            outsb = constp.tile([128, NCHUNK * D], F32, tag="outsb")
            epst = constp.tile([128, 1], F32, tag="epst")
            nc.vector.memset(epst[:], LN_EPS)
            gts = [constp.tile([128, K], BF16, tag=f"gt{i}", name=f"gt{i}")
                   for i in range(6)]
            tile_idx = [0]
            ev_idx = [0]
            cp_idx = [0]
            if variant != "full":
                nc.vector.memset(outsb[:], 0.0)
            if with_b1:
                ones = constp.tile([1, 128], F32R, tag="ones")
                nc.vector.memset(ones[:], 1.0)
                b1sb = constp.tile([1, D * K], F32R, tag="b1sb")
                nc.sync.dma_start(out=b1sb[:], in_=b1x[:])

            import contextlib
            loop_cm = tc.For_i(0, repeat, 1) if repeat > 1 else contextlib.nullcontext()

            def emit_tail(st):
                """LN scale/bias + gelu + dot for a chunk, emitted one chunk
                late so its (long-latency) stat joins never head-block the
                next chunk's phase-A evacuations on the ScalarE FIFO."""
                if st is None:
                    return
                c = st["c"]
                agW, rsW, nmW, hb_l = st["agW"], st["rsW"], st["nmW"], st["hb_l"]
                agV = agW.rearrange("p (n two) -> p n two", two=2)
                sdW = stp.tile([128, D], F32, tag="sdW")
                nc.scalar.activation(
                    sdW[:], agV[:, :, 1], AF.Sqrt, bias=epst[:, 0:1])
                nc.vector.reciprocal(rsW[:], sdW[:])
                nc.vector.scalar_tensor_tensor(
                    nmW[:], agV[:, :, 0], -1.0, rsW[:],
                    op0=ALU.mult, op1=ALU.mult)
                for d in range(D):
                    gt = gts[tile_idx[0] % 6]
                    tile_idx[0] += 1
                    if not with_affine:
                        nc.scalar.activation(
                            gt[:], hb_l[d][:], AF.Gelu,
                            bias=nmW[:, d:d + 1], scale=rsW[:, d:d + 1])
                    else:
                        hn = gp.tile([128, K], F32, tag="hn")
                        gat = gp.tile([128, K], F32, tag="gat")
                        bet = gp.tile([128, K], F32, tag="bet")
                        nc.sync.dma_start(out=gat[:], in_=garep[:, d, :])
                        nc.sync.dma_start(out=bet[:], in_=berep[:, d, :])
                        nc.scalar.activation(
                            hn[:], hb_l[d][:], AF.Identity,
                            bias=nmW[:, d:d + 1], scale=rsW[:, d:d + 1])
                        nc.vector.tensor_tensor(hn[:], hn[:], gat[:], op=ALU.mult)
                        nc.vector.tensor_tensor(hn[:], hn[:], bet[:], op=ALU.add)
                        nc.scalar.activation(gt[:], hn[:], AF.Gelu)
                    tmp = gp.tile([128, K], BF16, tag="tmp")
                    acc = outsb[:, c * D + d:c * D + d + 1]
                    nc.gpsimd.tensor_tensor(
                        tmp[:], gt[:], w2sb[:, d, :], op=ALU.mult)
                    nc.vector.reduce_sum(
                        acc, tmp[:], axis=mybir.AxisListType.X)

            with loop_cm:
              prev_st = None
              for c in range(NCHUNK):
                  pt = pts[c % 2]
                  # ---- phase A: pooled^T[h, hc, d, b] for this chunk ----
                  for gb in range(NG // GB):
                      xt = xp.tile([128, GB * H], BF16, tag="xt")
                      nc.sync.dma_start(out=xt[:], in_=x[c, gb])
                      if variant == "dma":
                          continue
                      for gpair in range(GB // 2):
                          pg = pgp.tile([128, 2, 512], F32, tag="pg")
                          if ilv:
                              mmseq = [(gg2, hc) for hc in range(HC)
                                       for gg2 in range(2)]
                          else:
                              mmseq = [(gg2, hc) for gg2 in range(2)
                                       for hc in range(HC)]
                          for gg2, hc in mmseq:
                              gi = gpair * 2 + gg2
                              nc.tensor.matmul(
                                  pg[:, gg2, hc * DJ:(hc + 1) * DJ],
                                  lhsT=xt[:,
                                          gi * H + hc * 128:gi * H + (hc + 1) * 128],
                                  rhs=mb[:],
                                  start=(hc == 0),
                                  stop=(hc == HC - 1),
                              )
                          # evacuate both groups with one instruction:
                          # pg[p, (gg,(hc d j))] -> pt[p, hc, d, 8*gp2+(gg,j)]
                          gp2 = gb * (GB // 2) + gpair
                          src = pg[:, :, 0:HC * DJ].rearrange(
                              "p g (hc d j) -> p hc d g j", hc=HC, d=D)
                          dst = pt[:, :, :, 8 * gp2:8 * gp2 + 8].rearrange(
                              "p hc d (g j) -> p hc d g j", g=2)
                          if ev_idx[0] % 8 < ev_act:
                              nc.scalar.copy(dst, src)
                          else:
                              nc.vector.tensor_copy(dst, src)
                          ev_idx[0] += 1

                  # ---- phase B: per-disease matmuls + PSUM evac + stats ----
                  if variant in ("dma", "pool"):
                      continue
                  agW = stp.tile([128, 2 * D], F32, tag="agW")
                  rsW = stp.tile([128, D], F32, tag="rsW")
                  nmW = stp.tile([128, D], F32, tag="nmW")
                  hb_l = []
                  for d0 in range(0, D, SG):
                      ds = list(range(d0, min(d0 + SG, D)))
                      nsg = len(ds)
                      hps_l = [hpp.tile([128, K], F32, tag="hps",
                                        name=f"hps{i}") for i in range(nsg)]
                      if ilv:
                          mmseq = [(i, hc) for hc in range(HC)
                                   for i in range(nsg)]
                      else:
                          mmseq = [(i, hc) for i in range(nsg)
                                   for hc in range(HC)]
                      for i, hc in mmseq:
                          d = ds[i]
                          nc.tensor.matmul(
                              hps_l[i][:],
                              lhsT=pt[:, hc, d, :],
                              rhs=w1sb[:, d, hc, :],
                              start=(hc == 0),
                              stop=(hc == HC - 1) and not with_b1,
                          )
                      for i, d in enumerate(ds):
                          if with_b1:
                              nc.tensor.matmul(
                                  hps_l[i][:],
                                  lhsT=ones[:],
                                  rhs=b1sb[:, d * K:(d + 1) * K],
                                  start=False,
                                  stop=True,
                              )
                          if variant == "mmonly":
                              continue
                          # single-hop PSUM evacuation: frees the bank for
                          # the next subgroup's matmuls without waiting on
                          # the LN/gelu chain; epilogue reads SBUF bf16.
                          hb = hbp.tile([128, K], BF16, tag="hb")
                          hb_l.append(hb)
                          if cp_idx[0] % 8 < cp_act:
                              nc.scalar.copy(hb[:], hps_l[i][:])
                          else:
                              nc.vector.tensor_copy(hb[:], hps_l[i][:])
                          cp_idx[0] += 1
                          bnst = stp.tile([128, 6], F32, tag="bnst")
                          nc.vector.bn_stats(bnst[:], hb[:])
                          nc.vector.bn_aggr(agW[:, 2 * d:2 * d + 2], bnst[:])
                  if variant == "mmonly":
                      continue
                  emit_tail(prev_st)
                  prev_st = {"c": c, "agW": agW, "rsW": rsW, "nmW": nmW,
                             "hb_l": hb_l}
              emit_tail(prev_st)

            nc.sync.dma_start(out=out[:], in_=outsb[:])

    return nc


def _host_prep(region_features, mask, W1, b1, gamma, beta, W2, b2):
    f32 = np.float32
    import ml_dtypes
    bf16 = ml_dtypes.bfloat16
    x = np.asarray(region_features)
    mask = np.asarray(mask)
    counts = mask.astype(np.int64).sum(axis=0)           # [D]
    ind = (counts > 0).astype(f32)                       # [D]

    # block-diag raw 0/1 mask: [(j,r)=116, (d,j)=56]
    mblk = np.zeros((128, DJ), dtype=bf16)
    mf = mask.astype(f32)                                # [R, D]
    for j in range(4):
        mblk[j * R:(j + 1) * R, :].reshape(R, D, 4)[:, :, j] = mf
    # w1 transposed to [p, (d, hc, k)] with h = hc*128 + p
    w1t = np.ascontiguousarray(
        np.asarray(W1, dtype=f32).reshape(D, HC, 128, K).transpose(2, 0, 1, 3)
    ).astype(bf16).reshape(128, D * HC * K)
    w2eff = (np.asarray(W2, dtype=f32) * ind[:, None]).astype(bf16)  # [D, K]
    w2rep = np.ascontiguousarray(
        np.broadcast_to(w2eff.reshape(1, D * K), (128, D * K)))
    b2eff = np.asarray(b2, dtype=f32) * ind               # added on host

    b1a = np.asarray(b1, dtype=f32)
    with_b1 = bool(np.any(b1a != 0.0))
    b1x = (b1a * counts.astype(f32)[:, None]).reshape(1, D * K) if with_b1 else None

    ga = np.asarray(gamma, dtype=f32)
    be = np.asarray(beta, dtype=f32)
    with_affine = bool(np.any(ga != 1.0) or np.any(be != 0.0))
    garep = berep = None
    if with_affine:
        garep = np.ascontiguousarray(np.broadcast_to(ga[None], (128, D, K)))
        berep = np.ascontiguousarray(np.broadcast_to(be[None], (128, D, K)))

    common = {"mblk": mblk, "w1t": w1t, "w2rep": w2rep}
    extra = {"b2eff": b2eff}
    if with_b1:
        common["b1x"] = b1x
    if with_affine:
        common["garep"] = garep
        common["berep"] = berep
    in_maps = []
    xb = np.asarray(x, dtype=bf16)                        # single bf16 plane
    for i in range(NCORES):
        m = dict(common)
        # b = c*128 + 4*g + j ; g = gb*GB + gi ; contiguous DMA layout
        xs = xb[i * BC:(i + 1) * BC].reshape(NCHUNK, NG // GB, GB, 4, R, H)
        xt_ = xs.transpose(0, 1, 3, 4, 2, 5).reshape(NCHUNK, NG // GB, JR, GB * H)
        xp_ = np.zeros((NCHUNK, NG // GB, 128, GB * H), dtype=bf16)
        xp_[:, :, 0:JR, :] = xt_
        m["x"] = xp_
        in_maps.append(m)
    return in_maps, with_b1, with_affine, extra


def kernel(region_features, mask, W1, b1, gamma, beta, W2, b2):
    from concourse.bass_utils import run_bass_kernel_spmd

    in_maps, with_b1, with_affine, extra = _host_prep(
        region_features, mask, W1, b1, gamma, beta, W2, b2
    )
    nc = build_nc(with_b1, with_affine)
    res = run_bass_kernel_spmd(nc, in_maps, list(range(NCORES)))
    outs = []
    for r in res.results:
        o = r["out"].reshape(128, NCHUNK, D).transpose(1, 0, 2).reshape(BC, D)
        outs.append(o)
    full = np.concatenate(outs, axis=0) + extra["b2eff"][None, :]
    return np.ascontiguousarray(full.astype(np.float32))


# revision 19
# speedup vs baseline: 1.3029x; 1.3029x over previous
"""Trainium2 Bass kernel for nn_DiseaseClassifier (segment_reduce).

reference semantics:
    m = mask.astype(f32); counts = m.sum(0)
    pooled = einsum('brh,rd->bdh', x, m) / max(counts,1)
    h = einsum('bdh,dhk->bdk', pooled, W1) + b1
    hn = LN(h) * gamma + beta ; g = gelu_exact(hn)
    preds = einsum('bdk,dk->bd', g, W2) + b2 ; preds[counts==0] = 0

Key algebraic facts used:
  * LayerNorm is scale-invariant, so the 1/count pooling divisor cancels
    (when b1 != 0 we add counts*b1 to the un-normalized pool-matmul output,
    which keeps the invariance exact).
  * b2 rides on the host; the counts==0 zeroing folds into W2.

Precision plan (correctness gate is rel err < 2e-2; measured 2.9e-3 in
host simulation): x, W1, pooled, gelu output and W2 all in bf16; all
matmul accumulation in f32 PSUM.  This halves the dominant HBM traffic
(x: 24.6 MB -> one bf16 plane) vs the fp32-accurate hi/lo bf16 split,
halves the phase-A stationary loads, and makes every matmul FWL-eligible
(128-col bf16 stationary => 2x weight-load bandwidth, load overlapped
with the previous matmul's moving phase; fp32/fp32r stationary loads
serialize and get no FWL).

Distribution: batch dim sharded over 8 NeuronCores (512 rows each); all
parameters replicated.  Inside each core:
  phase A: pool-matmul.  Stationary = x tiles [(4b,29r)=116, 128h] bf16,
           moving = 0/1 block-diag mask [116, 56=(14d,4j)] -> PSUM
           [128h, (2 groups)x(6hc,14d,4j)], evacuated by ScalarE/VectorE
           (alternating, 2 groups per instruction) into bf16
           pooledT [h, hc, d, b] layout.
  phase B: per-disease bf16 matmul pooledT[128h,128b] x W1[d][128h,384k]
           (6 h-chunks accumulated in PSUM), then bn_stats/bn_aggr ->
           Rsqrt -> single ScalarE activation gelu(scale*h+bias) with
           per-partition scale/bias doing the whole LayerNorm, then the
           g.W2 dot product on GpSimd (scalar_tensor_tensor with
           accum_out), keeping DVE free for stats + evac.

Engine budget per core-loop (modeled): PE ~100us (phase A 768 ldweights-
bound MMs + phase B 336 N=384 MMs), DMA ~69us (24.6MB bf16 x), ACT/DVE
~70us each (evac split + gelu / bn stats), GpSimd ~27us (dots).
"""

import os
import sys
import functools

for _p in ("/opt/trn_rl_repo", "/opt/pypackages"):
    if os.path.isdir(_p) and _p not in sys.path:
        sys.path.insert(0, _p)

import numpy as np

B, R, H, D = 4096, 29, 768, 14
K = H // 2            # 384
LN_EPS = 1e-5
NCORES = 8
BC = B // NCORES      # 512 batch rows per core
NCHUNK = BC // 128    # 4 chunks of 128 rows
NG = 32               # (4b,29r) groups per chunk
GB = 4                # groups per x-DMA batch
HC = H // 128         # 6 contraction chunks
JR = 4 * R            # 116 partitions for the pool matmul
DJ = D * 4            # 56 moving columns of the pool matmul


def _install_walrus_patches():
    """This walrus build supports only ONE sem wait per instruction
    ("Too many sync wait commands").  Split Tile-assigned multi-waits onto
    same-engine NoOps placed right before the instruction, and do the same
    for the TileContext tail drain."""
    from concourse import tile as _tile
    from concourse import mybir
    from concourse.vector_clock import ScopedClock

    if getattr(_tile.TileContext, "_ant_wait_split_patch", False):
        return
    _orig_commit = _tile.TileContext._commit_instruction

    def _patched_commit(self, inst, lazy_reg_writes=True):
        si = getattr(inst, "sync_info", None)
        if si is not None and si.on_wait and len(si.on_wait) > 1:
            waits = list(si.on_wait)
            inst.sync_info = mybir.SyncInfo(
                on_wait=[waits[-1]], on_update=list(si.on_update or [])
            )
            for w in waits[:-1]:
                nop = mybir.InstNoOp(
                    name=self.nc.get_next_instruction_name(), ins=[], outs=[]
                )
                nop.engine = inst.engine
                nop.sync_info = mybir.SyncInfo(on_wait=[w], on_update=[])
                self._add_instruction(nop)
        return _orig_commit(self, inst, lazy_reg_writes)

    def _patched_drain_and_barrier(self, tick_clock, wait_clock):
        drain_inst = self.nc.sync.drain()
        wait_clock.add_sem_waits(
            drain_inst.ins, ScopedClock({None: tick_clock.global_clock})
        )
        si = drain_inst.ins.sync_info
        if si is not None and si.on_wait and len(si.on_wait) > 1:
            waits = list(si.on_wait)
            drain_inst.ins.sync_info = mybir.SyncInfo(
                on_wait=[waits[0]], on_update=list(si.on_update or [])
            )
            for w in waits[1:]:
                d2 = self.nc.sync.drain()
                d2.ins.sync_info = mybir.SyncInfo(on_wait=[w], on_update=[])
        self.nc.all_engine_barrier()
        assert self.sems is not None
        popped = self.nc._tile_sem_poison_stack.pop()
        assert popped is self._sem_poison
        self.nc.clear_and_free_semaphores(list(self.sems.allocated().values()))
        self.nc.all_engine_barrier()

    _tile.TileContext._commit_instruction = _patched_commit
    _tile.TileContext._drain_and_barrier = _patched_drain_and_barrier
    _tile.TileContext._ant_wait_split_patch = True


@functools.lru_cache(maxsize=16)
def build_nc(with_b1: bool = False, with_affine: bool = False, repeat: int = 1,
             variant: str = "full", SG: int = 2, ev_act: int = 4,
             dot_gp: bool = True, cp_act: int = 6, ilv: bool = True,
             pipe_tail: bool = True, sr: bool = True):
    """Build the Bass program (identical on all 8 cores).

    ev_act: of every 8 two-group evacuations, this many go to ScalarE
    (the rest to VectorE).  cp_act: of every 8 hps->SBUF copies, this many
    go to ScalarE.  dot_gp: dot product = GpSimd mult + DVE reduce (else
    one DVE tensor_tensor_reduce)."""
    import concourse.bass as bass
    import concourse.mybir as mybir
    from concourse.tile import TileContext

    _install_walrus_patches()

    F32 = mybir.dt.float32
    F32R = mybir.dt.float32r
    BF16 = mybir.dt.bfloat16
    AF = mybir.ActivationFunctionType
    ALU = mybir.AluOpType

    nc = bass.Bass("TRN2", target_bir_lowering=False, debug=False,
                   num_devices=NCORES)

    x = nc.declare_dram_parameter("x", [NCHUNK, NG // GB, 128, GB * H],
                                  BF16, isOutput=False)
    # mask padded to 128 contraction rows (rows 116..127 zero): keeps the
    # stationary x tiles at the full 128 partitions so FWL triggers.
    mblk = nc.declare_dram_parameter("mblk", [128, DJ], BF16, isOutput=False)
    w1t = nc.declare_dram_parameter("w1t", [128, D * HC * K], BF16,
                                    isOutput=False)
    w2rep = nc.declare_dram_parameter("w2rep", [128, D * K], BF16,
                                      isOutput=False)
    if with_b1:
        b1x = nc.declare_dram_parameter("b1x", [1, D * K], F32R, isOutput=False)
    if with_affine:
        garep = nc.declare_dram_parameter("garep", [128, D, K], F32, isOutput=False)
        berep = nc.declare_dram_parameter("berep", [128, D, K], F32, isOutput=False)
    out = nc.declare_dram_parameter("out", [128, NCHUNK * D], F32, isOutput=True)

    with TileContext(nc) as tc:
        with (
            tc.tile_pool(name="const", bufs=1) as constp,
            tc.tile_pool(name="xin", bufs=6) as xp,
            tc.tile_pool(name="gly", bufs=3) as gp,
            tc.tile_pool(name="hb", bufs=32) as hbp,
            tc.tile_pool(name="st", bufs=4) as stp,
            tc.tile_pool(name="pg", bufs=2, space="PSUM") as pgp,
            tc.tile_pool(name="hp", bufs=4, space="PSUM") as hpp,
        ):
            mb = constp.tile([128, DJ], BF16, tag="mblk")
            nc.sync.dma_start(out=mb[:], in_=mblk[:])
            w1sb = constp.tile([128, D, HC, K], BF16, tag="w1sb")
            nc.sync.dma_start(
                out=w1sb.rearrange("p d hc k -> p (d hc k)"), in_=w1t[:])
            w2sb = constp.tile([128, D, K], BF16, tag="w2sb")
            nc.sync.dma_start(
                out=w2sb.rearrange("p d k -> p (d k)"), in_=w2rep[:])
            # double-buffered per chunk: breaks the WAR serialization
            # (phase-B reads of chunk c vs phase-A evac writes of chunk c+1)
            pts = [constp.tile([128, HC, D, 128], BF16, tag=f"pt{i}",
                               name=f"pt{i}") for i in range(2)]

            outsb = constp.tile([128, NCHUNK * D], F32, tag="outsb")
            epst = constp.tile([128, 1], F32, tag="epst")
            nc.vector.memset(epst[:], LN_EPS)
            gts = [constp.tile([128, K], BF16, tag=f"gt{i}", name=f"gt{i}")
                   for i in range(6)]
            tile_idx = [0]
            ev_idx = [0]
            cp_idx = [0]
            if variant != "full":
                nc.vector.memset(outsb[:], 0.0)
            if with_b1:
                ones = constp.tile([1, 128], F32R, tag="ones")
                nc.vector.memset(ones[:], 1.0)
                b1sb = constp.tile([1, D * K], F32R, tag="b1sb")
                nc.sync.dma_start(out=b1sb[:], in_=b1x[:])

            import contextlib
            loop_cm = (tc.For_i(0, repeat, 1, staggered_reset=sr)
                       if repeat > 1 else contextlib.nullcontext())

            def emit_tail(st):
                """LN scale/bias + gelu + dot for a chunk, emitted one chunk
                late so its (long-latency) stat joins never head-block the
                next chunk's phase-A evacuations on the ScalarE FIFO."""
                if st is None:
                    return
                c = st["c"]
                agW, rsW, nmW, hb_l = st["agW"], st["rsW"], st["nmW"], st["hb_l"]
                agV = agW.rearrange("p (n two) -> p n two", two=2)
                sdW = stp.tile([128, D], F32, tag="sdW")
                nc.scalar.activation(
                    sdW[:], agV[:, :, 1], AF.Sqrt, bias=epst[:, 0:1])
                nc.vector.reciprocal(rsW[:], sdW[:])
                nc.vector.scalar_tensor_tensor(
                    nmW[:], agV[:, :, 0], -1.0, rsW[:],
                    op0=ALU.mult, op1=ALU.mult)
                for d in range(D):
                    gt = gts[tile_idx[0] % 6]
                    tile_idx[0] += 1
                    if not with_affine:
                        nc.scalar.activation(
                            gt[:], hb_l[d][:], AF.Gelu,
                            bias=nmW[:, d:d + 1], scale=rsW[:, d:d + 1])
                    else:
                        hn = gp.tile([128, K], F32, tag="hn")
                        gat = gp.tile([128, K], F32, tag="gat")
                        bet = gp.tile([128, K], F32, tag="bet")
                        nc.sync.dma_start(out=gat[:], in_=garep[:, d, :])
                        nc.sync.dma_start(out=bet[:], in_=berep[:, d, :])
                        nc.scalar.activation(
                            hn[:], hb_l[d][:], AF.Identity,
                            bias=nmW[:, d:d + 1], scale=rsW[:, d:d + 1])
                        nc.vector.tensor_tensor(hn[:], hn[:], gat[:], op=ALU.mult)
                        nc.vector.tensor_tensor(hn[:], hn[:], bet[:], op=ALU.add)
                        nc.scalar.activation(gt[:], hn[:], AF.Gelu)
                    tmp = gp.tile([128, K], BF16, tag="tmp")
                    acc = outsb[:, c * D + d:c * D + d + 1]
                    nc.gpsimd.tensor_tensor(
                        tmp[:], gt[:], w2sb[:, d, :], op=ALU.mult)
                    nc.vector.reduce_sum(
                        acc, tmp[:], axis=mybir.AxisListType.X)

            with loop_cm:
              prev_st = None
              for c in range(NCHUNK):
                  pt = pts[c % 2]
                  # ---- phase A: pooled^T[h, hc, d, b] for this chunk ----
                  for gb in range(NG // GB):
                      xt = xp.tile([128, GB * H], BF16, tag="xt")
                      nc.sync.dma_start(out=xt[:], in_=x[c, gb])
                      if variant == "dma":
                          continue
                      for gpair in range(GB // 2):
                          pg = pgp.tile([128, 2, 512], F32, tag="pg")
                          if ilv:
                              mmseq = [(gg2, hc) for hc in range(HC)
                                       for gg2 in range(2)]
                          else:
                              mmseq = [(gg2, hc) for gg2 in range(2)
                                       for hc in range(HC)]
                          for gg2, hc in mmseq:
                              gi = gpair * 2 + gg2
                              nc.tensor.matmul(
                                  pg[:, gg2, hc * DJ:(hc + 1) * DJ],
                                  lhsT=xt[:,
                                          gi * H + hc * 128:gi * H + (hc + 1) * 128],
                                  rhs=mb[:],
                                  start=(hc == 0),
                                  stop=(hc == HC - 1),
                              )
                          # evacuate both groups with one instruction:
                          # pg[p, (gg,(hc d j))] -> pt[p, hc, d, 8*gp2+(gg,j)]
                          gp2 = gb * (GB // 2) + gpair
                          src = pg[:, :, 0:HC * DJ].rearrange(
                              "p g (hc d j) -> p hc d g j", hc=HC, d=D)
                          dst = pt[:, :, :, 8 * gp2:8 * gp2 + 8].rearrange(
                              "p hc d (g j) -> p hc d g j", g=2)
                          if ev_idx[0] % 8 < ev_act:
                              nc.scalar.copy(dst, src)
                          else:
                              nc.vector.tensor_copy(dst, src)
                          ev_idx[0] += 1

                  # ---- phase B: per-disease matmuls + PSUM evac + stats ----
                  if variant in ("dma", "pool"):
                      continue
                  agW = stp.tile([128, 2 * D], F32, tag="agW")
                  rsW = stp.tile([128, D], F32, tag="rsW")
                  nmW = stp.tile([128, D], F32, tag="nmW")
                  hb_l = []
                  for d0 in range(0, D, SG):
                      ds = list(range(d0, min(d0 + SG, D)))
                      nsg = len(ds)
                      hps_l = [hpp.tile([128, K], F32, tag="hps",
                                        name=f"hps{i}") for i in range(nsg)]
                      if ilv:
                          mmseq = [(i, hc) for hc in range(HC)
                                   for i in range(nsg)]
                      else:
                          mmseq = [(i, hc) for i in range(nsg)
                                   for hc in range(HC)]
                      for i, hc in mmseq:
                          d = ds[i]
                          nc.tensor.matmul(
                              hps_l[i][:],
                              lhsT=pt[:, hc, d, :],
                              rhs=w1sb[:, d, hc, :],
                              start=(hc == 0),
                              stop=(hc == HC - 1) and not with_b1,
                          )
                      for i, d in enumerate(ds):
                          if with_b1:
                              nc.tensor.matmul(
                                  hps_l[i][:],
                                  lhsT=ones[:],
                                  rhs=b1sb[:, d * K:(d + 1) * K],
                                  start=False,
                                  stop=True,
                              )
                          if variant == "mmonly":
                              continue
                          # single-hop PSUM evacuation: frees the bank for
                          # the next subgroup's matmuls without waiting on
                          # the LN/gelu chain; epilogue reads SBUF bf16.
                          hb = hbp.tile([128, K], BF16, tag="hb")
                          hb_l.append(hb)
                          if cp_idx[0] % 8 < cp_act:
                              nc.scalar.copy(hb[:], hps_l[i][:])
                          else:
                              nc.vector.tensor_copy(hb[:], hps_l[i][:])
                          cp_idx[0] += 1
                          bnst = stp.tile([128, 6], F32, tag="bnst")
                          nc.vector.bn_stats(bnst[:], hb[:])
                          nc.vector.bn_aggr(agW[:, 2 * d:2 * d + 2], bnst[:])
                  if variant == "mmonly":
                      continue
                  st = {"c": c, "agW": agW, "rsW": rsW, "nmW": nmW,
                        "hb_l": hb_l}
                  if pipe_tail:
                      emit_tail(prev_st)
                      prev_st = st
                  else:
                      emit_tail(st)
              emit_tail(prev_st)

            nc.sync.dma_start(out=out[:], in_=outsb[:])

    return nc


def _host_prep(region_features, mask, W1, b1, gamma, beta, W2, b2):
    f32 = np.float32
    import ml_dtypes
    bf16 = ml_dtypes.bfloat16
    x = np.asarray(region_features)
    mask = np.asarray(mask)
    counts = mask.astype(np.int64).sum(axis=0)           # [D]
    ind = (counts > 0).astype(f32)                       # [D]

    # block-diag raw 0/1 mask: [(j,r)=116, (d,j)=56]
    mblk = np.zeros((128, DJ), dtype=bf16)
    mf = mask.astype(f32)                                # [R, D]
    for j in range(4):
        mblk[j * R:(j + 1) * R, :].reshape(R, D, 4)[:, :, j] = mf
    # w1 transposed to [p, (d, hc, k)] with h = hc*128 + p
    w1t = np.ascontiguousarray(
        np.asarray(W1, dtype=f32).reshape(D, HC, 128, K).transpose(2, 0, 1, 3)
    ).astype(bf16).reshape(128, D * HC * K)
    w2eff = (np.asarray(W2, dtype=f32) * ind[:, None]).astype(bf16)  # [D, K]
    w2rep = np.ascontiguousarray(
        np.broadcast_to(w2eff.reshape(1, D * K), (128, D * K)))
    b2eff = np.asarray(b2, dtype=f32) * ind               # added on host

    b1a = np.asarray(b1, dtype=f32)
    with_b1 = bool(np.any(b1a != 0.0))
    b1x = (b1a * counts.astype(f32)[:, None]).reshape(1, D * K) if with_b1 else None

    ga = np.asarray(gamma, dtype=f32)
    be = np.asarray(beta, dtype=f32)
    with_affine = bool(np.any(ga != 1.0) or np.any(be != 0.0))
    garep = berep = None
    if with_affine:
        garep = np.ascontiguousarray(np.broadcast_to(ga[None], (128, D, K)))
        berep = np.ascontiguousarray(np.broadcast_to(be[None], (128, D, K)))

    common = {"mblk": mblk, "w1t": w1t, "w2rep": w2rep}
    extra = {"b2eff": b2eff}
    if with_b1:
        common["b1x"] = b1x
    if with_affine:
        common["garep"] = garep
        common["berep"] = berep
    in_maps = []
    xb = np.asarray(x, dtype=bf16)                        # single bf16 plane
    for i in range(NCORES):
        m = dict(common)
        # b = c*128 + 4*g + j ; g = gb*GB + gi ; contiguous DMA layout
        xs = xb[i * BC:(i + 1) * BC].reshape(NCHUNK, NG // GB, GB, 4, R, H)
        xt_ = xs.transpose(0, 1, 3, 4, 2, 5).reshape(NCHUNK, NG // GB, JR, GB * H)
        xp_ = np.zeros((NCHUNK, NG // GB, 128, GB * H), dtype=bf16)
        xp_[:, :, 0:JR, :] = xt_
        m["x"] = xp_
        in_maps.append(m)
    return in_maps, with_b1, with_affine, extra


def kernel(region_features, mask, W1, b1, gamma, beta, W2, b2):
    from concourse.bass_utils import run_bass_kernel_spmd

    in_maps, with_b1, with_affine, extra = _host_prep(
        region_features, mask, W1, b1, gamma, beta, W2, b2
    )
    nc = build_nc(with_b1, with_affine)
    res = run_bass_kernel_spmd(nc, in_maps, list(range(NCORES)))
    outs = []
    for r in res.results:
        o = r["out"].reshape(128, NCHUNK, D).transpose(1, 0, 2).reshape(BC, D)
        outs.append(o)
    full = np.concatenate(outs, axis=0) + extra["b2eff"][None, :]
    return np.ascontiguousarray(full.astype(np.float32))


# revision 23
# speedup vs baseline: 1.4284x; 1.0963x over previous
"""Trainium2 Bass kernel for nn_DiseaseClassifier (segment_reduce).

reference semantics:
    m = mask.astype(f32); counts = m.sum(0)
    pooled = einsum('brh,rd->bdh', x, m) / max(counts,1)
    h = einsum('bdh,dhk->bdk', pooled, W1) + b1
    hn = LN(h) * gamma + beta ; g = gelu_exact(hn)
    preds = einsum('bdk,dk->bd', g, W2) + b2 ; preds[counts==0] = 0

Key algebraic facts used:
  * LayerNorm is scale-invariant, so the 1/count pooling divisor cancels
    (when b1 != 0 we add counts*b1 to the un-normalized pool-matmul output,
    which keeps the invariance exact).
  * b2 rides on the host; the counts==0 zeroing folds into W2.

Precision plan (correctness gate: rel err < 2e-2; measured 4.5e-3 on HW):
x, W1, pooled, h, gelu output and W2 all bf16; matmul accumulation f32
PSUM.  Halves the dominant HBM traffic vs the fp32-accurate hi/lo split
and makes every matmul FWL-eligible (128-col bf16 stationary).

Distribution: batch dim sharded over 8 NeuronCores (512 rows each); all
parameters replicated and loaded to SBUF once, outside the timed loop.
Per core, per 128-row chunk:
  phase A: pool-matmul.  Stationary = x tiles [(4b,29r)=116+12pad, 128h]
           bf16, moving = 0/1 block-diag mask [128, 56=(14d,4j)] -> PSUM
           [128h, 2x(6hc,14d,4j)], evacuated (2 groups per instruction,
           ScalarE/VectorE alternating) into bf16 pooledT [h, hc, d, b],
           double-buffered per chunk to break the WAR serialization
           against the next chunk's evacuations.
  phase B: per-disease bf16 matmul pooledT[128h,128b] x W1[d][128h,384k]
           (6 h-chunks accumulated in PSUM), copied at once to SBUF bf16
           (single-hop PSUM evacuation so the LN/gelu chain never blocks
           the array), then bn_stats/bn_aggr -> sqrt/recip -> one gelu
           activation per disease with per-partition scale/bias doing
           the whole LayerNorm, then the g.W2 dot as GpSimd multiply +
           VectorE reduce.

Scheduling (the big wins beyond dtype/layout):
  * Phase A is ldweights+drain bound (~91ns per 56-col matmul): its
    weight loads hide under phase B's long 384-col streams by weaving
    chunk c-1's phase-B matmuls between chunk c's pool matmuls (abi=2:
    one B matmul per two A matmuls, generators interleaved at emission).
  * The per-chunk epilogue tail (sqrt->gelu->dot) is emitted one chunk
    late and one disease at a time, so its long cross-engine latency
    chain never head-blocks phase-A evacuations on the strict-FIFO
    ScalarE queue.
  * For_i(staggered_reset=True) avoids the all-engine barrier per
    repeat-loop iteration.

Measured (axon TRN2, per core, drift-cancelled repeat-loop): ~139-151us
per exec (process-to-process variance ~10%), vs 299us for the hi/lo fp32
baseline.  Segment times (same-process): x DMA alone 64.5us (~380GB/s,
at the HBM roofline), +pool matmuls 71-76us, +phase B matmuls 132us.
"""

import os
import sys
import functools

for _p in ("/opt/trn_rl_repo", "/opt/pypackages"):
    if os.path.isdir(_p) and _p not in sys.path:
        sys.path.insert(0, _p)

import numpy as np

B, R, H, D = 4096, 29, 768, 14
K = H // 2            # 384
LN_EPS = 1e-5
NCORES = 8
BC = B // NCORES      # 512 batch rows per core
NCHUNK = BC // 128    # 4 chunks of 128 rows
NG = 32               # (4b,29r) groups per chunk
GB = 4                # groups per x-DMA batch
HC = H // 128         # 6 contraction chunks
JR = 4 * R            # 116 partitions for the pool matmul
DJ = D * 4            # 56 moving columns of the pool matmul


def _install_walrus_patches():
    """This walrus build supports only ONE sem wait per instruction
    ("Too many sync wait commands").  Split Tile-assigned multi-waits onto
    same-engine NoOps placed right before the instruction, and do the same
    for the TileContext tail drain."""
    from concourse import tile as _tile
    from concourse import mybir
    from concourse.vector_clock import ScopedClock

    if getattr(_tile.TileContext, "_ant_wait_split_patch", False):
        return
    _orig_commit = _tile.TileContext._commit_instruction

    def _patched_commit(self, inst, lazy_reg_writes=True):
        si = getattr(inst, "sync_info", None)
        if si is not None and si.on_wait and len(si.on_wait) > 1:
            waits = list(si.on_wait)
            inst.sync_info = mybir.SyncInfo(
                on_wait=[waits[-1]], on_update=list(si.on_update or [])
            )
            for w in waits[:-1]:
                nop = mybir.InstNoOp(
                    name=self.nc.get_next_instruction_name(), ins=[], outs=[]
                )
                nop.engine = inst.engine
                nop.sync_info = mybir.SyncInfo(on_wait=[w], on_update=[])
                self._add_instruction(nop)
        return _orig_commit(self, inst, lazy_reg_writes)

    def _patched_drain_and_barrier(self, tick_clock, wait_clock):
        drain_inst = self.nc.sync.drain()
        wait_clock.add_sem_waits(
            drain_inst.ins, ScopedClock({None: tick_clock.global_clock})
        )
        si = drain_inst.ins.sync_info
        if si is not None and si.on_wait and len(si.on_wait) > 1:
            waits = list(si.on_wait)
            drain_inst.ins.sync_info = mybir.SyncInfo(
                on_wait=[waits[0]], on_update=list(si.on_update or [])
            )
            for w in waits[1:]:
                d2 = self.nc.sync.drain()
                d2.ins.sync_info = mybir.SyncInfo(on_wait=[w], on_update=[])
        self.nc.all_engine_barrier()
        assert self.sems is not None
        popped = self.nc._tile_sem_poison_stack.pop()
        assert popped is self._sem_poison
        self.nc.clear_and_free_semaphores(list(self.sems.allocated().values()))
        self.nc.all_engine_barrier()

    _tile.TileContext._commit_instruction = _patched_commit
    _tile.TileContext._drain_and_barrier = _patched_drain_and_barrier
    _tile.TileContext._ant_wait_split_patch = True


@functools.lru_cache(maxsize=16)
def build_nc(with_b1: bool = False, with_affine: bool = False, repeat: int = 1,
             variant: str = "full", SG: int = 2, ev_act: int = 4,
             dot_gp: bool = True, cp_act: int = 8, ilv: bool = True,
             pipe_tail: bool = True, sr: bool = True, abi: int = 2):
    """Build the Bass program (identical on all 8 cores).

    ev_act: of every 8 two-group evacuations, this many go to ScalarE
    (the rest to VectorE).  cp_act: of every 8 hps->SBUF copies, this many
    go to ScalarE.  dot_gp: dot product = GpSimd mult + DVE reduce (else
    one DVE tensor_tensor_reduce)."""
    import concourse.bass as bass
    import concourse.mybir as mybir
    from concourse.tile import TileContext

    _install_walrus_patches()

    F32 = mybir.dt.float32
    F32R = mybir.dt.float32r
    BF16 = mybir.dt.bfloat16
    AF = mybir.ActivationFunctionType
    ALU = mybir.AluOpType

    nc = bass.Bass("TRN2", target_bir_lowering=False, debug=False,
                   num_devices=NCORES)

    x = nc.declare_dram_parameter("x", [NCHUNK, NG // GB, 128, GB * H],
                                  BF16, isOutput=False)
    # mask padded to 128 contraction rows (rows 116..127 zero): keeps the
    # stationary x tiles at the full 128 partitions so FWL triggers.
    mblk = nc.declare_dram_parameter("mblk", [128, DJ], BF16, isOutput=False)
    w1t = nc.declare_dram_parameter("w1t", [128, D * HC * K], BF16,
                                    isOutput=False)
    w2rep = nc.declare_dram_parameter("w2rep", [128, D * K], BF16,
                                      isOutput=False)
    if with_b1:
        b1x = nc.declare_dram_parameter("b1x", [1, D * K], F32R, isOutput=False)
    if with_affine:
        garep = nc.declare_dram_parameter("garep", [128, D, K], F32, isOutput=False)
        berep = nc.declare_dram_parameter("berep", [128, D, K], F32, isOutput=False)
    out = nc.declare_dram_parameter("out", [128, NCHUNK * D], F32, isOutput=True)

    with TileContext(nc) as tc:
        with (
            tc.tile_pool(name="const", bufs=1) as constp,
            tc.tile_pool(name="xin", bufs=6) as xp,
            tc.tile_pool(name="gly", bufs=3) as gp,
            tc.tile_pool(name="hb", bufs=32) as hbp,
            tc.tile_pool(name="st", bufs=4) as stp,
            tc.tile_pool(name="pg", bufs=2, space="PSUM") as pgp,
            tc.tile_pool(name="hp", bufs=4, space="PSUM") as hpp,
        ):
            mb = constp.tile([128, DJ], BF16, tag="mblk")
            nc.sync.dma_start(out=mb[:], in_=mblk[:])
            w1sb = constp.tile([128, D, HC, K], BF16, tag="w1sb")
            nc.sync.dma_start(
                out=w1sb.rearrange("p d hc k -> p (d hc k)"), in_=w1t[:])
            w2sb = constp.tile([128, D, K], BF16, tag="w2sb")
            nc.sync.dma_start(
                out=w2sb.rearrange("p d k -> p (d k)"), in_=w2rep[:])
            # double-buffered per chunk: breaks the WAR serialization
            # (phase-B reads of chunk c vs phase-A evac writes of chunk c+1)
            pts = [constp.tile([128, HC, D, 128], BF16, tag=f"pt{i}",
                               name=f"pt{i}") for i in range(2)]

            outsb = constp.tile([128, NCHUNK * D], F32, tag="outsb")
            epst = constp.tile([128, 1], F32, tag="epst")
            nc.vector.memset(epst[:], LN_EPS)
            gts = [constp.tile([128, K], BF16, tag=f"gt{i}", name=f"gt{i}")
                   for i in range(6)]
            tile_idx = [0]
            ev_idx = [0]
            cp_idx = [0]
            if variant != "full":
                nc.vector.memset(outsb[:], 0.0)
            if with_b1:
                ones = constp.tile([1, 128], F32R, tag="ones")
                nc.vector.memset(ones[:], 1.0)
                b1sb = constp.tile([1, D * K], F32R, tag="b1sb")
                nc.sync.dma_start(out=b1sb[:], in_=b1x[:])

            import contextlib
            loop_cm = (tc.For_i(0, repeat, 1, staggered_reset=sr)
                       if repeat > 1 else contextlib.nullcontext())

            def tail_gen(st):
                """LN scale/bias + gelu + dot for a chunk; yields per
                disease so the driver can weave it between phase-A work
                instead of bursting 14 gelus onto the ScalarE FIFO."""
                if st is None or not st:
                    return
                c = st["c"]
                agW, rsW, nmW, hb_l = st["agW"], st["rsW"], st["nmW"], st["hb_l"]
                agV = agW.rearrange("p (n two) -> p n two", two=2)
                sdW = stp.tile([128, D], F32, tag="sdW")
                nc.scalar.activation(
                    sdW[:], agV[:, :, 1], AF.Sqrt, bias=epst[:, 0:1])
                nc.vector.reciprocal(rsW[:], sdW[:])
                nc.vector.scalar_tensor_tensor(
                    nmW[:], agV[:, :, 0], -1.0, rsW[:],
                    op0=ALU.mult, op1=ALU.mult)
                for d in range(D):
                    gt = gts[tile_idx[0] % 6]
                    tile_idx[0] += 1
                    if not with_affine:
                        nc.scalar.activation(
                            gt[:], hb_l[d][:], AF.Gelu,
                            bias=nmW[:, d:d + 1], scale=rsW[:, d:d + 1])
                    else:
                        hn = gp.tile([128, K], F32, tag="hn")
                        gat = gp.tile([128, K], F32, tag="gat")
                        bet = gp.tile([128, K], F32, tag="bet")
                        nc.sync.dma_start(out=gat[:], in_=garep[:, d, :])
                        nc.sync.dma_start(out=bet[:], in_=berep[:, d, :])
                        nc.scalar.activation(
                            hn[:], hb_l[d][:], AF.Identity,
                            bias=nmW[:, d:d + 1], scale=rsW[:, d:d + 1])
                        nc.vector.tensor_tensor(hn[:], hn[:], gat[:], op=ALU.mult)
                        nc.vector.tensor_tensor(hn[:], hn[:], bet[:], op=ALU.add)
                        nc.scalar.activation(gt[:], hn[:], AF.Gelu)
                    tmp = gp.tile([128, K], BF16, tag="tmp")
                    acc = outsb[:, c * D + d:c * D + d + 1]
                    nc.gpsimd.tensor_tensor(
                        tmp[:], gt[:], w2sb[:, d, :], op=ALU.mult)
                    nc.vector.reduce_sum(
                        acc, tmp[:], axis=mybir.AxisListType.X)
                    yield

            def emit_tail(st):
                for _ in tail_gen(st):
                    pass

            def a_block(c):
                """Phase A for chunk c; yields after each pool matmul."""
                pt = pts[c % 2]
                for gb in range(NG // GB):
                    xt = xp.tile([128, GB * H], BF16, tag="xt")
                    nc.sync.dma_start(out=xt[:], in_=x[c, gb])
                    if variant == "dma":
                        continue
                    for gpair in range(GB // 2):
                        pg = pgp.tile([128, 2, 512], F32, tag="pg")
                        if ilv:
                            mmseq = [(gg2, hc) for hc in range(HC)
                                     for gg2 in range(2)]
                        else:
                            mmseq = [(gg2, hc) for gg2 in range(2)
                                     for hc in range(HC)]
                        for gg2, hc in mmseq:
                            gi = gpair * 2 + gg2
                            nc.tensor.matmul(
                                pg[:, gg2, hc * DJ:(hc + 1) * DJ],
                                lhsT=xt[:,
                                        gi * H + hc * 128:gi * H + (hc + 1) * 128],
                                rhs=mb[:],
                                start=(hc == 0),
                                stop=(hc == HC - 1),
                            )
                            yield
                        # evacuate both groups with one instruction:
                        # pg[p, (gg,(hc d j))] -> pt[p, hc, d, 8*gp2+(gg,j)]
                        gp2 = gb * (GB // 2) + gpair
                        src = pg[:, :, 0:HC * DJ].rearrange(
                            "p g (hc d j) -> p hc d g j", hc=HC, d=D)
                        dst = pt[:, :, :, 8 * gp2:8 * gp2 + 8].rearrange(
                            "p hc d (g j) -> p hc d g j", g=2)
                        if ev_idx[0] % 8 < ev_act:
                            nc.scalar.copy(dst, src)
                        else:
                            nc.vector.tensor_copy(dst, src)
                        ev_idx[0] += 1

            def b_block(c, st):
                """Phase B matmuls + PSUM evac + stats for chunk c; yields
                after each matmul.  Fills st with the tail state."""
                pt = pts[c % 2]
                agW = stp.tile([128, 2 * D], F32, tag="agW")
                rsW = stp.tile([128, D], F32, tag="rsW")
                nmW = stp.tile([128, D], F32, tag="nmW")
                hb_l = []
                st.update({"c": c, "agW": agW, "rsW": rsW, "nmW": nmW,
                           "hb_l": hb_l})
                for d0 in range(0, D, SG):
                    ds = list(range(d0, min(d0 + SG, D)))
                    nsg = len(ds)
                    hps_l = [hpp.tile([128, K], F32, tag="hps",
                                      name=f"hps{i}") for i in range(nsg)]
                    if ilv:
                        mmseq = [(i, hc) for hc in range(HC)
                                 for i in range(nsg)]
                    else:
                        mmseq = [(i, hc) for i in range(nsg)
                                 for hc in range(HC)]
                    for i, hc in mmseq:
                        d = ds[i]
                        nc.tensor.matmul(
                            hps_l[i][:],
                            lhsT=pt[:, hc, d, :],
                            rhs=w1sb[:, d, hc, :],
                            start=(hc == 0),
                            stop=(hc == HC - 1) and not with_b1,
                        )
                        yield
                    for i, d in enumerate(ds):
                        if with_b1:
                            nc.tensor.matmul(
                                hps_l[i][:],
                                lhsT=ones[:],
                                rhs=b1sb[:, d * K:(d + 1) * K],
                                start=False,
                                stop=True,
                            )
                        if variant == "mmonly":
                            continue
                        # single-hop PSUM evacuation: frees the bank fast;
                        # the epilogue reads the SBUF bf16 copy.
                        hb = hbp.tile([128, K], BF16, tag="hb")
                        hb_l.append(hb)
                        if cp_idx[0] % 8 < cp_act:
                            nc.scalar.copy(hb[:], hps_l[i][:])
                        else:
                            nc.vector.tensor_copy(hb[:], hps_l[i][:])
                        cp_idx[0] += 1
                        bnst = stp.tile([128, 6], F32, tag="bnst")
                        nc.vector.bn_stats(bnst[:], hb[:])
                        nc.vector.bn_aggr(agW[:, 2 * d:2 * d + 2], bnst[:])

            def drain(gen, n=None):
                k = 0
                while n is None or k < n:
                    try:
                        next(gen)
                    except StopIteration:
                        return False
                    k += 1
                return True

            with loop_cm:
              if abi and variant == "full":
                  # Software pipeline with PE interleave: phase B matmuls of
                  # chunk c-1 (long 384-col moving) are woven between phase A
                  # matmuls of chunk c (short 56-col moving, ldweights/drain
                  # bound) so the array stream hides the weight loads; the
                  # chunk c-2 epilogue tail is woven in one disease at a time.
                  b_pend = None      # B-block generator in flight (chunk c-1)
                  b_st = None
                  t_pend = None      # tail generator in flight (chunk c-2)
                  for c in range(NCHUNK):
                      ag = a_block(c)
                      alive_a = True
                      na = 0
                      while alive_a:
                          alive_a = drain(ag, abi)
                          na += abi
                          if b_pend is not None and not drain(b_pend, 1):
                              b_pend = None
                          if na % 12 == 0 and t_pend is not None:
                              if not drain(t_pend, 1):
                                  t_pend = None
                      if b_pend is not None:
                          drain(b_pend)
                      if t_pend is not None:
                          drain(t_pend)
                      t_pend = tail_gen(b_st) if b_st else None
                      b_st = {}
                      b_pend = b_block(c, b_st)
                  drain(b_pend)
                  if t_pend is not None:
                      drain(t_pend)
                  emit_tail(b_st)
              else:
                  prev_st = None
                  for c in range(NCHUNK):
                      drain(a_block(c))
                      # ---- phase B + stats ----
                      if variant in ("dma", "pool"):
                          continue
                      st = {}
                      drain(b_block(c, st))
                      if variant == "mmonly":
                          continue
                      if pipe_tail:
                          emit_tail(prev_st)
                          prev_st = st
                      else:
                          emit_tail(st)
                  emit_tail(prev_st)

            nc.sync.dma_start(out=out[:], in_=outsb[:])

    return nc


def _host_prep(region_features, mask, W1, b1, gamma, beta, W2, b2):
    f32 = np.float32
    import ml_dtypes
    bf16 = ml_dtypes.bfloat16
    x = np.asarray(region_features)
    mask = np.asarray(mask)
    counts = mask.astype(np.int64).sum(axis=0)           # [D]
    ind = (counts > 0).astype(f32)                       # [D]

    # block-diag raw 0/1 mask: [(j,r)=116, (d,j)=56]
    mblk = np.zeros((128, DJ), dtype=bf16)
    mf = mask.astype(f32)                                # [R, D]
    for j in range(4):
        mblk[j * R:(j + 1) * R, :].reshape(R, D, 4)[:, :, j] = mf
    # w1 transposed to [p, (d, hc, k)] with h = hc*128 + p
    w1t = np.ascontiguousarray(
        np.asarray(W1, dtype=f32).reshape(D, HC, 128, K).transpose(2, 0, 1, 3)
    ).astype(bf16).reshape(128, D * HC * K)
    w2eff = (np.asarray(W2, dtype=f32) * ind[:, None]).astype(bf16)  # [D, K]
    w2rep = np.ascontiguousarray(
        np.broadcast_to(w2eff.reshape(1, D * K), (128, D * K)))
    b2eff = np.asarray(b2, dtype=f32) * ind               # added on host

    b1a = np.asarray(b1, dtype=f32)
    with_b1 = bool(np.any(b1a != 0.0))
    b1x = (b1a * counts.astype(f32)[:, None]).reshape(1, D * K) if with_b1 else None

    ga = np.asarray(gamma, dtype=f32)
    be = np.asarray(beta, dtype=f32)
    with_affine = bool(np.any(ga != 1.0) or np.any(be != 0.0))
    garep = berep = None
    if with_affine:
        garep = np.ascontiguousarray(np.broadcast_to(ga[None], (128, D, K)))
        berep = np.ascontiguousarray(np.broadcast_to(be[None], (128, D, K)))

    common = {"mblk": mblk, "w1t": w1t, "w2rep": w2rep}
    extra = {"b2eff": b2eff}
    if with_b1:
        common["b1x"] = b1x
    if with_affine:
        common["garep"] = garep
        common["berep"] = berep
    in_maps = []
    xb = np.asarray(x, dtype=bf16)                        # single bf16 plane
    for i in range(NCORES):
        m = dict(common)
        # b = c*128 + 4*g + j ; g = gb*GB + gi ; contiguous DMA layout
        xs = xb[i * BC:(i + 1) * BC].reshape(NCHUNK, NG // GB, GB, 4, R, H)
        xt_ = xs.transpose(0, 1, 3, 4, 2, 5).reshape(NCHUNK, NG // GB, JR, GB * H)
        xp_ = np.zeros((NCHUNK, NG // GB, 128, GB * H), dtype=bf16)
        xp_[:, :, 0:JR, :] = xt_
        m["x"] = xp_
        in_maps.append(m)
    return in_maps, with_b1, with_affine, extra


def kernel(region_features, mask, W1, b1, gamma, beta, W2, b2):
    from concourse.bass_utils import run_bass_kernel_spmd

    in_maps, with_b1, with_affine, extra = _host_prep(
        region_features, mask, W1, b1, gamma, beta, W2, b2
    )
    nc = build_nc(with_b1, with_affine)
    res = run_bass_kernel_spmd(nc, in_maps, list(range(NCORES)))
    outs = []
    for r in res.results:
        o = r["out"].reshape(128, NCHUNK, D).transpose(1, 0, 2).reshape(BC, D)
        outs.append(o)
    full = np.concatenate(outs, axis=0) + extra["b2eff"][None, :]
    return np.ascontiguousarray(full.astype(np.float32))
